# revision 3
# baseline (speedup 1.0000x reference)
"""nn_NewNormalizationUnit kernel: full inputs -> full outputs on 8 TRN2 cores.

Dev version — imports trnkernel (will be inlined for the final deliverable).
"""
import sys
sys.path.insert(0, "/opt/trn_rl_repo")
sys.path.insert(0, "/root/problem")
import numpy as np

N_CORES = 8
LAST_EXEC_NS = None
LAST_RESULTS = None
_CACHED_NC = {}


def _install_ntff_hook():
    """Provide antenv.axon_hooks (missing in this image) so trace=True works."""
    import types
    if "antenv.axon_hooks" in sys.modules:
        return
    try:
        sys.path.insert(0, "/root/.axon_site")
        from trn_agent_boot.trn_boot import _ntff_profile_via_ctypes
        hook = _ntff_profile_via_ctypes("/opt/axon/libaxon_pjrt.so")
    except Exception:
        hook = None
    mod = types.ModuleType("antenv.axon_hooks")
    mod.get_axon_ntff_profile_hook = lambda: hook
    mod.set_axon_ntff_profile_hook = lambda h: None
    sys.modules["antenv.axon_hooks"] = mod


def _get_nc(rows):
    if rows not in _CACHED_NC:
        import trnkernel
        _CACHED_NC[rows] = trnkernel.build_nc(rows)
    return _CACHED_NC[rows]


def kernel(P):
    global LAST_EXEC_NS, LAST_RESULTS
    import trnkernel
    from concourse.bass_utils import run_bass_kernel_spmd

    P = np.ascontiguousarray(np.asarray(P, dtype=np.float32))
    B = P.shape[0]
    assert B % N_CORES == 0
    rows = B // N_CORES

    import os
    trace = os.environ.get("KERNEL_TRACE", "0") == "1"
    if trace:
        _install_ntff_hook()
    nc = _get_nc(rows)
    in_maps = [trnkernel.host_inputs(P[c * rows:(c + 1) * rows]) for c in range(N_CORES)]
    res = run_bass_kernel_spmd(nc, in_maps, list(range(N_CORES)), trace=trace)
    LAST_EXEC_NS = res.exec_time_ns
    LAST_RESULTS = res

    outs = []
    for name in ["p_norm", "exp_adj", "sticky", "overflow", "shift_amt"]:
        outs.append(np.concatenate([res.results[c][name] for c in range(N_CORES)], axis=0))
    return tuple(outs)
